# revision 2
# baseline (speedup 1.0000x reference)
"""Trainium2 Bass kernel v2 for nn_Attention1D (GroupNorm -> QKV -> MHA ->
proj -> residual), B=4, C=512, L=2048, H=8, D=64, 32 groups.

Sharding: 8 cores, core i handles batch i//2 and heads [4*(i%2), 4*(i%2)+4).
Host sums the two partial projections per batch and adds proj bias + residual.

The backend executes the per-core program at a roughly flat ~50-80us per
instruction with ALL engines AND DMA queues serialized into one stream
(measured: a 64-matmul body times identically whether ops are spread across
engines or not, and interleaved PE+Act bodies cost the sum, not the max).
So the only lever is TOTAL instruction count; v2 trims it vs v1 (977->909):
  - Softmax normalization is batched: one reciprocal + one raw copy per
    head, then a single DRAM-broadcast round trip and ONE [128, 2x2048]
    multiply for all four heads (v1: per-head-half recip + 2 DMAs + mul,
    16 DMAs and 8 reciprocals more).
  - QKV biases ride one [128, 2048]-wide tensor_scalar per output tile
    (v1: eight [128,1024] adds); proj copies are 4x [128,2048] (v1: 16).
  - Small constants (gG, ones) are packed into the f32r wp input; bq/bk/
    gamma/beta/bv8 share one f32 sm input: 6 input DMAs total (v1: 7+).
Instruction floors that resist further cuts: matmul moving-operand free
size is hard-capped at 512 elements (ucode assert s3d3_mm_num_elements),
so S and PV need 64 matmuls per head each; exp needs 16 [128,2048] reads
per head (PSUM-capacity bound). Attention = 512 of the 674 matmuls.
Attention dataflow per head (all matmuls f32r, PSUM fp32):
  S^T[lk,lq] = KT.T @ QT ; P = exp(S*scale) (no max-sub: |S*scale| < ~6)
  [O^T; s] = [V | 1].T @ P (ones column makes softmax sums a free 65th row)
  O^T *= 1/s (batched, partition-broadcast via DRAM round-trip DMA)
"""

import sys

sys.path.insert(0, "/opt/trn_rl_repo")

import numpy as np

import concourse.bass as bass
import concourse.tile as tile
from concourse import mybir
from concourse.bass_utils import run_bass_kernel_spmd

B, C, L = 4, 512, 2048
H, D = 8, 64
GROUPS = 32
EPS = 1e-5
NCORES = 8
HPC = H // 2  # heads per core = 4

F32 = mybir.dt.float32
F32R = mybir.dt.float32r
BF16 = mybir.dt.bfloat16


def reduce_waits(nc):
    """Drop sem-ge-imm waits already implied by an earlier wait in the same
    execution stream (engine, or DMA queue for DMACopy): engines and queues
    retire their instructions in order and semaphores only count upward, so
    once a stream has waited S>=v, any later wait S>=v'<=v is a no-op."""
    reducible = ("EngineType.PE", "EngineType.DVE", "EngineType.Activation",
                 "EngineType.Pool")
    dropped = 0
    for blk in nc.m.functions[0].blocks:
        marks = {}
        for inst in blk.instructions:
            eng = str(inst.engine)
            if eng not in reducible or inst.opcode in (
                    "DMACopy", "TensorLoad", "TensorSave", "DmaTransposeAnt"):
                continue
            wm = marks.setdefault(eng, {})
            si = inst.sync_info
            if si and si.on_wait:
                kept = []
                for w in si.on_wait:
                    if (w.sync_type == "semaphore"
                            and w.wait_mode == "sem-ge-imm"
                            and w.wait_reg is None):
                        if wm.get(w.id, -1) >= w.wait_value:
                            dropped += 1
                            continue
                        wm[w.id] = w.wait_value
                    kept.append(w)
                si.on_wait = kept
    return dropped


def split_waits(nc, max_waits=1, drop_own=()):
    """walrus codegen accepts at most one sync-wait per instruction: merge
    redundant sem-ge-imm waits (max value per semaphore), drop waits on the
    engine's own semaphore, and push any excess onto NoOp chains."""
    own_sem = {
        "EngineType.PE": "PE_",
        "EngineType.DVE": "DVE_",
        "EngineType.Activation": "Activation_",
        "EngineType.Pool": "Pool_",
    }
    cnt = 0
    for blk in nc.m.functions[0].blocks:
        out = []
        for inst in blk.instructions:
            si = inst.sync_info
            if si and si.on_wait and len(si.on_wait) > max_waits:
                eng_short = str(inst.engine).split(".")[-1]
                own = own_sem.get(str(inst.engine)) if eng_short in drop_own else None
                merged = {}
                rest = []
                for w in si.on_wait:
                    if (own and w.sync_type == "semaphore"
                            and w.wait_mode == "sem-ge-imm"
                            and w.ant_name and w.ant_name.startswith(own)):
                        continue
                    if (w.sync_type == "semaphore"
                            and w.wait_mode == "sem-ge-imm"
                            and w.wait_reg is None):
                        key = w.id
                        if key not in merged or merged[key].wait_value < w.wait_value:
                            merged[key] = w
                    else:
                        rest.append(w)
                waits = list(merged.values()) + rest
                if not waits:
                    si.on_wait = []
                    out.append(inst)
                    continue
                si.on_wait = [waits[-1]]
                for w in waits[:-1]:
                    cnt += 1
                    out.append(mybir.InstNoOp(
                        name=f"I-wsplit-{cnt}",
                        engine=inst.engine,
                        sync_info=mybir.SyncInfo(on_wait=[w], on_update=[]),
                    ))
            out.append(inst)
        blk.instructions = out
    return cnt


def build_nc(apply_split=True, reps=1, drop_own=('Activation',)):
    nc = bass.Bass()
    AF = mybir.ActivationFunctionType

    x_in = nc.dram_tensor("x_in", [128, 4, L], F32R, kind="ExternalInput")
    wqkv = nc.dram_tensor("wqkv", [128, 4, 768], F32R, kind="ExternalInput")
    wp = nc.dram_tensor("wp", [128, 2, 584], F32R, kind="ExternalInput")
    # sm = [bq(2) | bk(2) | gam(4) | bet(4) | pad(2) | bv8(2048)]
    sm = nc.dram_tensor("sm", [128, 2062], F32, kind="ExternalInput")
    gG2 = nc.dram_tensor("gG2", [8, 128], F32R, kind="ExternalInput")
    r_dram = nc.dram_tensor("r_dram", [4, 2048], F32)
    out_d = nc.dram_tensor("out", [128, 4, L], F32, kind="ExternalOutput")

    with tile.TileContext(nc) as tc:
        ctx_lp = nc.allow_low_precision(
            reason="f32r SBUF tiles feed f32r matmuls; PSUM accumulation "
                   "stays fp32")
        ctx_lp.__enter__()
        with tc.tile_pool(name="const", bufs=1) as const, \
             tc.tile_pool(name="acts", bufs=1) as acts, \
             tc.tile_pool(name="work", bufs=3) as work, \
             tc.tile_pool(name="ps_a", bufs=1, space="PSUM") as ps_a, \
             tc.tile_pool(name="ps_b", bufs=1, space="PSUM") as ps_b:

            for _rep in range(reps):
                # ---- load constants + input ----
                wqkv_sb = const.tile([128, 4, 768], F32R)
                wp_sb = const.tile([128, 2, 584], F32R)
                sm_sb = const.tile([128, 2062], F32)
                G2_sb = const.tile([8, 128], F32R)
                eps_sb = const.tile([128, 1], F32)
                nc.sync.dma_start(wqkv_sb[:], wqkv[:])
                nc.sync.dma_start(wp_sb[:], wp[:])
                nc.sync.dma_start(sm_sb[:], sm[:])
                nc.sync.dma_start(G2_sb[:], gG2[:])
                nc.vector.memset(eps_sb[:], EPS)
                wq_sb = wqkv_sb[:].rearrange("p a (w c) -> p a w c", w=3)[:, :, 0, :]
                wk_sb = wqkv_sb[:].rearrange("p a (w c) -> p a w c", w=3)[:, :, 1, :]
                wv_sb = wqkv_sb[:].rearrange("p a (w c) -> p a w c", w=3)[:, :, 2, :]
                bq_sb = sm_sb[:, 0:2]
                bk_sb = sm_sb[:, 2:4]
                gam_sb = sm_sb[:, 4:8]
                bet_sb = sm_sb[:, 8:12]
                bv8_sb = sm_sb[:, 14:2062]
                gG_sb = wp_sb[:, 0, 512:520]
                ones_sb = wp_sb[:, 0, 520:584]

                X = acts.tile([128, 4, L], F32R)
                nc.sync.dma_start(X[:], x_in[:])
                Xf = X[:].bitcast(F32)

                # ---- GroupNorm ----
                # per-channel sum / sum-of-squares via ACT accum_out
                stats8 = work.tile([128, 8], F32R, name="stats8")
                gn_scr = work.tile([128, 2048], F32, tag="ot",
                                   name="gn_scr", bufs=2)
                for j in range(4):
                    nc.scalar.activation(gn_scr[:], Xf[:, j, :], AF.Identity,
                                         accum_out=stats8[:, j:j + 1])
                    nc.scalar.activation(gn_scr[:], Xf[:, j, :], AF.Square,
                                         accum_out=stats8[:, 4 + j:5 + j])
                # group-reduce over the 16 partitions of each group
                psg = ps_b.tile([8, 8], F32, tag="pv", name="psum_g")
                nc.tensor.matmul(psg[:], gG_sb, stats8[:], start=True,
                                 stop=True)
                bc_in = work.tile([8, 8], F32R, name="bc_in")
                t8 = work.tile([8, 4], F32, name="t8")
                t8b = work.tile([8, 4], F32, name="t8b")
                inv_n = 1.0 / (16 * 2048)
                nc.vector.tensor_scalar_mul(bc_in[:, 0:4], psg[:, 0:4], inv_n)
                nc.vector.tensor_scalar_mul(t8[:], psg[:, 4:8], inv_n)
                nc.vector.tensor_mul(t8b[:], bc_in[:, 0:4].bitcast(F32),
                                     bc_in[:, 0:4].bitcast(F32))
                nc.vector.tensor_sub(t8[:], t8[:], t8b[:])
                nc.scalar.activation(t8[:], t8[:], AF.Sqrt, bias=eps_sb[0:8, :])
                nc.vector.reciprocal(bc_in[:, 4:8], t8[:])
                # broadcast to channels
                psbc = ps_b.tile([128, 8], F32, tag="pv", name="psum_bc")
                nc.tensor.matmul(psbc[:], G2_sb[:], bc_in[:], start=True,
                                 stop=True)
                A4 = work.tile([128, 4], F32, name="A4")
                B4 = work.tile([128, 4], F32, name="B4")
                nc.vector.tensor_mul(A4[:], psbc[:, 4:8], gam_sb)
                nc.vector.tensor_mul(B4[:], psbc[:, 0:4], A4[:])
                nc.vector.tensor_sub(B4[:], bet_sb, B4[:])
                X2 = X
                for j in range(4):
                    nc.vector.tensor_scalar(
                        out=X2[:, j, :], in0=Xf[:, j, :],
                        scalar1=A4[:, j:j + 1], scalar2=B4[:, j:j + 1],
                        op0=mybir.AluOpType.mult, op1=mybir.AluOpType.add)

                # ---- QKV (all channel-major; 16 matmuls + 1 bias add per
                # 128-channel tile, PSUM [128,2048] alternating pools) ----
                QT = acts.tile([128, 2, L], F32R)
                KT = acts.tile([128, 2, L], F32R)
                tiles = [(wq_sb, 0, QT[:, 0, :], bq_sb[:, 0:1]),
                         (wq_sb, 1, QT[:, 1, :], bq_sb[:, 1:2]),
                         (wk_sb, 0, KT[:, 0, :], bk_sb[:, 0:1]),
                         (wk_sb, 1, KT[:, 1, :], bk_sb[:, 1:2])]
                for ti, (wsb, blk, dst, bias) in enumerate(tiles):
                    pool = ps_a if ti % 2 == 0 else ps_b
                    tag = "s" if ti % 2 == 0 else "pv"
                    pq = pool.tile([128, 2048], F32, tag=tag, name="psq")
                    for n4 in range(4):
                        for kc in range(4):
                            nc.tensor.matmul(
                                pq[:, n4 * 512:(n4 + 1) * 512],
                                wsb[:, kc, blk * 128:(blk + 1) * 128],
                                X2[:, kc, n4 * 512:(n4 + 1) * 512],
                                start=(kc == 0), stop=(kc == 3))
                    nc.vector.tensor_scalar_add(dst, pq[:], bias)

                # ---- V, computed directly transposed: [lk, 256 ch] with
                # a ones column per head (65th) for the softmax-sum trick ----
                Vt = acts.tile([128, 16, 260], F32R)
                Vt4 = Vt[:].rearrange("p t (h m) -> p t h m", m=65)
                nc.vector.tensor_copy(
                    Vt4[:, :, :, 64],
                    ones_sb.rearrange("p (t h) -> p t h", t=16))
                for g2 in range(2):
                    pool = ps_a if g2 % 2 == 0 else ps_b
                    tag = "s" if g2 % 2 == 0 else "pv"
                    pv_ = pool.tile([128, 2048], F32, tag=tag, name="psv")
                    for l8 in range(8):
                        lk = g2 * 8 + l8
                        for kc in range(4):
                            nc.tensor.matmul(
                                pv_[:, l8 * 256:(l8 + 1) * 256],
                                X2[:, kc, lk * 128:(lk + 1) * 128],
                                wv_sb[:, kc, :], start=(kc == 0),
                                stop=(kc == 3))
                    nc.vector.tensor_add(
                        Vt4[:, g2 * 8:(g2 + 1) * 8, :, 0:64],
                        pv_[:].rearrange("p (a h m) -> p a h m", a=8, h=4),
                        bv8_sb.rearrange("p (a h m) -> p a h m", a=8, h=4))

                # ---- attention ----
                OT = work.tile([128, 2, L], F32R, tag="vo", name="OT",
                               bufs=1)
                rr = work.tile([1, 4, 2048], F32, name="rr", bufs=1)
                for u in range(HPC):
                    blk, poff = u // 2, 64 * (u % 2)
                    pvp = ps_b.tile([128, 2048], F32, tag="pv", name="pvp")
                    # software-pipelined: PV runs one round behind scores+exp
                    P_prev = None
                    for lk in range(17):
                        P_cur = None
                        if lk >= 1:
                            lp = lk - 1
                            for c2 in range(4):
                                nc.tensor.matmul(
                                    pvp[0:65, c2 * 512:(c2 + 1) * 512],
                                    Vt[:, lp, u * 65:u * 65 + 65],
                                    P_prev[:, c2 * 512:(c2 + 1) * 512],
                                    start=(lp == 0), stop=(lp == 15))
                        if lk < 16:
                            S = ps_a.tile([128, 2048], F32, tag="s", name="S")
                            for c2 in range(4):
                                c0 = c2 * 512
                                nc.tensor.matmul(
                                    S[:, c0:c0 + 512],
                                    KT[poff:poff + 64, blk,
                                       lk * 128:(lk + 1) * 128],
                                    QT[poff:poff + 64, blk, c0:c0 + 512],
                                    start=True, stop=True)
                            P_cur = work.tile([128, 2048], F32R, tag="P",
                                              name="P", bufs=2)
                            nc.scalar.activation(
                                P_cur[:], S[:], AF.Exp,
                                scale=float(1.0 / np.sqrt(D)))
                        P_prev = P_cur
                    # per-head: softmax sums reciprocal + raw copy; the
                    # broadcasted divide happens once for all heads below
                    nc.vector.reciprocal(rr[:, u, :], pvp[64:65, :])
                    nc.vector.tensor_copy(OT[poff:poff + 64, blk, :],
                                          pvp[0:64, :])
                # batched normalization: one DRAM round trip broadcasts
                # rr[head, lq] across the 64 d-partitions of each head
                nc.sync.dma_start(r_dram[:], rr[:])
                src = r_dram[:]
                Rb = work.tile([128, 2, 2048], F32, name="Rb", bufs=1)
                for hl in range(2):
                    rbc = bass.AP(tensor=src.tensor,
                                  offset=src.offset + hl * 2048,
                                  ap=[[0, 64], [4096, 2], [1, 2048]])
                    nc.sync.dma_start(Rb[hl * 64:(hl + 1) * 64, :, :], rbc)
                nc.vector.tensor_mul(OT[:], OT[:].bitcast(F32), Rb[:])

                # ---- projection (partial over this core's 256 channels) ----
                for mt in range(4):
                    pool = ps_a if mt % 2 == 0 else ps_b
                    tag = "s" if mt % 2 == 0 else "pv"
                    po = pool.tile([128, 2048], F32, tag=tag, name="po")
                    for n4 in range(4):
                        for kc in range(2):
                            nc.tensor.matmul(
                                po[:, n4 * 512:(n4 + 1) * 512],
                                wp_sb[:, kc, mt * 128:(mt + 1) * 128],
                                OT[:, kc, n4 * 512:(n4 + 1) * 512],
                                start=(kc == 0), stop=(kc == 1))
                    ot = work.tile([128, 2048], F32, tag="ot", name="ot",
                                   bufs=2)
                    nc.vector.tensor_copy(ot[:], po[:])
                    nc.sync.dma_start(out_d[:, mt, :], ot[:])

        ctx_lp.__exit__(None, None, None)

    if apply_split:
        split_waits(nc, drop_own=drop_own)
    return nc


_CACHE = {}


def _get_nc():
    if "nc" not in _CACHE:
        _CACHE["nc"] = build_nc()
    return _CACHE["nc"]


def _core_inputs(i, x, gamma, beta, w_qkv, b_qkv, w_proj, b_proj):
    b, j0 = i // 2, i % 2
    heads = [HPC * j0 + k for k in range(HPC)]
    # Q/K row order: blk-major, within blk: head pair x d
    qidx = np.array([heads[blk * 2 + p // 64] * 64 + p % 64
                     for blk in range(2) for p in range(128)])
    kidx = qidx + C
    vidx = np.array([2 * C + heads[n // 64] * 64 + n % 64 for n in range(256)])
    pcol = np.array([heads[cc // 64] * 64 + cc % 64 for cc in range(256)])

    f32 = np.float32
    wq_a = np.ascontiguousarray(
        w_qkv[qidx].T.reshape(4, 128, 256).transpose(1, 0, 2)).astype(f32)
    wk_a = np.ascontiguousarray(
        w_qkv[kidx].T.reshape(4, 128, 256).transpose(1, 0, 2)).astype(f32)
    wv_a = np.ascontiguousarray(
        w_qkv[vidx].T.reshape(4, 128, 256).transpose(1, 0, 2)).astype(f32)
    gG = (np.arange(128)[:, None] // 16 == np.arange(8)[None, :]).astype(f32)
    sm = np.empty((128, 2062), f32)
    sm[:, 0:2] = b_qkv[qidx].reshape(2, 128).T
    sm[:, 2:4] = b_qkv[kidx].reshape(2, 128).T
    sm[:, 4:8] = gamma.reshape(4, 128).T
    sm[:, 8:12] = beta.reshape(4, 128).T
    sm[:, 12:14] = 0.0
    sm[:, 14:2062] = np.tile(b_qkv[vidx], (128, 8))
    m = {
        "x_in": np.ascontiguousarray(
            x[b].reshape(4, 128, L).transpose(1, 0, 2)).astype(f32),
        "wqkv": np.concatenate([wq_a, wk_a, wv_a], axis=2),
        "wp": np.concatenate([
            np.ascontiguousarray(
                w_proj[:, pcol].T.reshape(2, 128, 512).transpose(1, 0, 2)
            ).astype(f32),
            np.stack([gG, np.zeros_like(gG)], axis=1),
            np.stack([np.ones((128, 64), f32),
                      np.zeros((128, 64), f32)], axis=1)], axis=2),
        "sm": sm,
        "gG2": (np.arange(8)[:, None]
                == np.arange(128)[None, :] // 16).astype(f32),
    }
    return m


def kernel(x, gamma, beta, w_qkv, b_qkv, w_proj, b_proj, _trace=False):
    x = np.asarray(x, dtype=np.float32)
    gamma = np.asarray(gamma, dtype=np.float32)
    beta = np.asarray(beta, dtype=np.float32)
    w_qkv = np.asarray(w_qkv, dtype=np.float32)
    b_qkv = np.asarray(b_qkv, dtype=np.float32)
    w_proj = np.asarray(w_proj, dtype=np.float32)
    b_proj = np.asarray(b_proj, dtype=np.float32)

    nc = _get_nc()
    in_maps = [_core_inputs(i, x, gamma, beta, w_qkv, b_qkv, w_proj, b_proj)
               for i in range(NCORES)]
    res = run_bass_kernel_spmd(nc, in_maps, list(range(NCORES)),
                               trace=_trace)
    out = np.empty((B, C, L), dtype=np.float32)
    for b in range(B):
        acc = x[b] + b_proj[:, None]
        for j0 in range(2):
            part = res.results[2 * b + j0]["out"]  # [128, 4, L]
            acc = acc + part.transpose(1, 0, 2).reshape(C, L)
        out[b] = acc
    if _trace:
        return out, res
    return out


# revision 4
# speedup vs baseline: 1.2843x; 1.2843x over previous
"""Trainium2 Bass kernel v2 for nn_Attention1D (GroupNorm -> QKV -> MHA ->
proj -> residual), B=4, C=512, L=2048, H=8, D=64, 32 groups.

Sharding: 8 cores, core i handles batch i//2 and heads [4*(i%2), 4*(i%2)+4).
Host sums the two partial projections per batch and adds proj bias + residual.

The backend executes the per-core program at a roughly flat ~50-80us per
instruction with ALL engines AND DMA queues serialized into one stream
(measured: a 64-matmul body times identically whether ops are spread across
engines or not, and interleaved PE+Act bodies cost the sum, not the max).
So the only lever is TOTAL instruction count; v2 trims it vs v1 (977->909):
  - Softmax normalization is batched: one reciprocal + one raw copy per
    head, then a single DRAM-broadcast round trip and ONE [128, 2x2048]
    multiply for all four heads (v1: per-head-half recip + 2 DMAs + mul,
    16 DMAs and 8 reciprocals more).
  - QKV biases ride one [128, 2048]-wide tensor_scalar per output tile
    (v1: eight [128,1024] adds); proj copies are 4x [128,2048] (v1: 16).
  - Small constants (gG, ones) are packed into the f32r wp input; bq/bk/
    gamma/beta/bv8 share one f32 sm input: 6 input DMAs total (v1: 7+).
  - V is computed channel-major (32 N=512 matmuls instead of 64 N=256
    in the transposed layout), stored contiguously to DRAM in bf16, and
    transposed back by 16 canonical 2D xbar DMA transposes
    ([256,128] -> [128,256]); one strided copy re-groups the columns into
    per-head [64 | ones] 65-column blocks. P (exp output) is bf16 too, so
    PV runs as a uniform bf16 matmul (rel err 1.1e-4, budget 2e-2).
    bf16 matmuls emit one Ldweights each (+256 instructions), but those
    measure ~free on this backend; head-to-head interleaved A/B showed
    this V path ~28% faster than computing V transposed in f32r.
Instruction floors that resist further cuts: matmul moving-operand free
size is hard-capped at 512 elements (ucode assert s3d3_mm_num_elements),
so S and PV need 64 matmuls per head each; exp needs 16 [128,2048] reads
per head (PSUM-capacity bound). Attention = 512 of the 642 matmuls.
Attention dataflow per head (all matmuls f32r, PSUM fp32):
  S^T[lk,lq] = KT.T @ QT ; P = exp(S*scale) (no max-sub: |S*scale| < ~6)
  [O^T; s] = [V | 1].T @ P (ones column makes softmax sums a free 65th row)
  O^T *= 1/s (batched, partition-broadcast via DRAM round-trip DMA)
"""

import sys

sys.path.insert(0, "/opt/trn_rl_repo")

import numpy as np

import concourse.bass as bass
import concourse.tile as tile
from concourse import mybir
from concourse.bass_utils import run_bass_kernel_spmd

B, C, L = 4, 512, 2048
H, D = 8, 64
GROUPS = 32
EPS = 1e-5
NCORES = 8
HPC = H // 2  # heads per core = 4

F32 = mybir.dt.float32
F32R = mybir.dt.float32r
BF16 = mybir.dt.bfloat16


def reduce_waits(nc):
    """Drop sem-ge-imm waits already implied by an earlier wait in the same
    execution stream (engine, or DMA queue for DMACopy): engines and queues
    retire their instructions in order and semaphores only count upward, so
    once a stream has waited S>=v, any later wait S>=v'<=v is a no-op."""
    reducible = ("EngineType.PE", "EngineType.DVE", "EngineType.Activation",
                 "EngineType.Pool")
    dropped = 0
    for blk in nc.m.functions[0].blocks:
        marks = {}
        for inst in blk.instructions:
            eng = str(inst.engine)
            if eng not in reducible or inst.opcode in (
                    "DMACopy", "TensorLoad", "TensorSave", "DmaTransposeAnt"):
                continue
            wm = marks.setdefault(eng, {})
            si = inst.sync_info
            if si and si.on_wait:
                kept = []
                for w in si.on_wait:
                    if (w.sync_type == "semaphore"
                            and w.wait_mode == "sem-ge-imm"
                            and w.wait_reg is None):
                        if wm.get(w.id, -1) >= w.wait_value:
                            dropped += 1
                            continue
                        wm[w.id] = w.wait_value
                    kept.append(w)
                si.on_wait = kept
    return dropped


def split_waits(nc, max_waits=1, drop_own=()):
    """walrus codegen accepts at most one sync-wait per instruction: merge
    redundant sem-ge-imm waits (max value per semaphore), drop waits on the
    engine's own semaphore, and push any excess onto NoOp chains."""
    own_sem = {
        "EngineType.PE": "PE_",
        "EngineType.DVE": "DVE_",
        "EngineType.Activation": "Activation_",
        "EngineType.Pool": "Pool_",
    }
    cnt = 0
    for blk in nc.m.functions[0].blocks:
        out = []
        for inst in blk.instructions:
            si = inst.sync_info
            if si and si.on_wait and len(si.on_wait) > max_waits:
                eng_short = str(inst.engine).split(".")[-1]
                own = own_sem.get(str(inst.engine)) if eng_short in drop_own else None
                merged = {}
                rest = []
                for w in si.on_wait:
                    if (own and w.sync_type == "semaphore"
                            and w.wait_mode == "sem-ge-imm"
                            and w.ant_name and w.ant_name.startswith(own)):
                        continue
                    if (w.sync_type == "semaphore"
                            and w.wait_mode == "sem-ge-imm"
                            and w.wait_reg is None):
                        key = w.id
                        if key not in merged or merged[key].wait_value < w.wait_value:
                            merged[key] = w
                    else:
                        rest.append(w)
                waits = list(merged.values()) + rest
                if not waits:
                    si.on_wait = []
                    out.append(inst)
                    continue
                si.on_wait = [waits[-1]]
                for w in waits[:-1]:
                    cnt += 1
                    out.append(mybir.InstNoOp(
                        name=f"I-wsplit-{cnt}",
                        engine=inst.engine,
                        sync_info=mybir.SyncInfo(on_wait=[w], on_update=[]),
                    ))
            out.append(inst)
        blk.instructions = out
    return cnt


def build_nc(apply_split=True, reps=1, drop_own=('Activation',)):
    nc = bass.Bass()
    AF = mybir.ActivationFunctionType

    x_in = nc.dram_tensor("x_in", [128, 4, L], F32R, kind="ExternalInput")
    wqkv = nc.dram_tensor("wqkv", [128, 4, 768], F32R, kind="ExternalInput")
    wp = nc.dram_tensor("wp", [128, 2, 584], F32R, kind="ExternalInput")
    # sm = [bq(2) | bk(2) | gam(4) | bet(4) | pad(2) | bv8(2048)]
    sm = nc.dram_tensor("sm", [128, 2062], F32, kind="ExternalInput")
    gG2 = nc.dram_tensor("gG2", [8, 128], F32R, kind="ExternalInput")
    r_dram = nc.dram_tensor("r_dram", [4, 2048], F32)
    v_dram = nc.dram_tensor("v_dram", [256, 2048], BF16)
    out_d = nc.dram_tensor("out", [128, 4, L], F32, kind="ExternalOutput")

    with tile.TileContext(nc) as tc:
        ctx_lp = nc.allow_low_precision(
            reason="f32r SBUF tiles feed f32r matmuls; PSUM accumulation "
                   "stays fp32")
        ctx_lp.__enter__()
        with tc.tile_pool(name="const", bufs=1) as const, \
             tc.tile_pool(name="acts", bufs=1) as acts, \
             tc.tile_pool(name="work", bufs=3) as work, \
             tc.tile_pool(name="ps_a", bufs=1, space="PSUM") as ps_a, \
             tc.tile_pool(name="ps_b", bufs=1, space="PSUM") as ps_b:

            for _rep in range(reps):
                # ---- load constants + input ----
                wqkv_sb = const.tile([128, 4, 768], F32R)
                wp_sb = const.tile([128, 2, 584], F32R)
                sm_sb = const.tile([128, 2062], F32)
                G2_sb = const.tile([8, 128], F32R)
                eps_sb = const.tile([128, 1], F32)
                nc.sync.dma_start(wqkv_sb[:], wqkv[:])
                nc.sync.dma_start(wp_sb[:], wp[:])
                nc.sync.dma_start(sm_sb[:], sm[:])
                nc.sync.dma_start(G2_sb[:], gG2[:])
                nc.vector.memset(eps_sb[:], EPS)
                wq_sb = wqkv_sb[:].rearrange("p a (w c) -> p a w c", w=3)[:, :, 0, :]
                wk_sb = wqkv_sb[:].rearrange("p a (w c) -> p a w c", w=3)[:, :, 1, :]
                wv_sb = wqkv_sb[:].rearrange("p a (w c) -> p a w c", w=3)[:, :, 2, :]
                bq_sb = sm_sb[:, 0:2]
                bk_sb = sm_sb[:, 2:4]
                gam_sb = sm_sb[:, 4:8]
                bet_sb = sm_sb[:, 8:12]
                bv8_sb = sm_sb[:, 14:2062]
                gG_sb = wp_sb[:, 0, 512:520]
                ones_sb = wp_sb[:, 0, 520:584]

                X = acts.tile([128, 4, L], F32R)
                nc.sync.dma_start(X[:], x_in[:])
                Xf = X[:].bitcast(F32)

                # ---- GroupNorm ----
                # per-channel sum / sum-of-squares via ACT accum_out
                stats8 = work.tile([128, 8], F32R, name="stats8")
                gn_scr = work.tile([128, 2048], F32, tag="ot",
                                   name="gn_scr", bufs=2)
                for j in range(4):
                    nc.scalar.activation(gn_scr[:], Xf[:, j, :], AF.Identity,
                                         accum_out=stats8[:, j:j + 1])
                    nc.scalar.activation(gn_scr[:], Xf[:, j, :], AF.Square,
                                         accum_out=stats8[:, 4 + j:5 + j])
                # group-reduce over the 16 partitions of each group
                psg = ps_b.tile([8, 8], F32, tag="pv", name="psum_g")
                nc.tensor.matmul(psg[:], gG_sb, stats8[:], start=True,
                                 stop=True)
                bc_in = work.tile([8, 8], F32R, name="bc_in")
                t8 = work.tile([8, 4], F32, name="t8")
                t8b = work.tile([8, 4], F32, name="t8b")
                inv_n = 1.0 / (16 * 2048)
                nc.vector.tensor_scalar_mul(bc_in[:, 0:4], psg[:, 0:4], inv_n)
                nc.vector.tensor_scalar_mul(t8[:], psg[:, 4:8], inv_n)
                nc.vector.tensor_mul(t8b[:], bc_in[:, 0:4].bitcast(F32),
                                     bc_in[:, 0:4].bitcast(F32))
                nc.vector.tensor_sub(t8[:], t8[:], t8b[:])
                nc.scalar.activation(t8[:], t8[:], AF.Sqrt, bias=eps_sb[0:8, :])
                nc.vector.reciprocal(bc_in[:, 4:8], t8[:])
                # broadcast to channels
                psbc = ps_b.tile([128, 8], F32, tag="pv", name="psum_bc")
                nc.tensor.matmul(psbc[:], G2_sb[:], bc_in[:], start=True,
                                 stop=True)
                A4 = work.tile([128, 4], F32, name="A4")
                B4 = work.tile([128, 4], F32, name="B4")
                nc.vector.tensor_mul(A4[:], psbc[:, 4:8], gam_sb)
                nc.vector.tensor_mul(B4[:], psbc[:, 0:4], A4[:])
                nc.vector.tensor_sub(B4[:], bet_sb, B4[:])
                X2 = X
                for j in range(4):
                    nc.vector.tensor_scalar(
                        out=X2[:, j, :], in0=Xf[:, j, :],
                        scalar1=A4[:, j:j + 1], scalar2=B4[:, j:j + 1],
                        op0=mybir.AluOpType.mult, op1=mybir.AluOpType.add)

                # ---- QKV (all channel-major; 16 matmuls + 1 bias add per
                # 128-channel tile, PSUM [128,2048] alternating pools) ----
                QT = acts.tile([128, 2, L], F32R)
                KT = acts.tile([128, 2, L], F32R)
                Vc = work.tile([128, 2, L], BF16, tag="rb", name="Vc",
                               bufs=1)
                bv_sb = sm_sb[:, 12:14]
                tiles = [(wq_sb, 0, QT[:, 0, :], bq_sb[:, 0:1]),
                         (wq_sb, 1, QT[:, 1, :], bq_sb[:, 1:2]),
                         (wk_sb, 0, KT[:, 0, :], bk_sb[:, 0:1]),
                         (wk_sb, 1, KT[:, 1, :], bk_sb[:, 1:2]),
                         (wv_sb, 0, Vc[:, 0, :], bv_sb[:, 0:1]),
                         (wv_sb, 1, Vc[:, 1, :], bv_sb[:, 1:2])]
                for ti, (wsb, blk, dst, bias) in enumerate(tiles):
                    pool = ps_a if ti % 2 == 0 else ps_b
                    tag = "s" if ti % 2 == 0 else "pv"
                    pq = pool.tile([128, 2048], F32, tag=tag, name="psq")
                    for n4 in range(4):
                        for kc in range(4):
                            nc.tensor.matmul(
                                pq[:, n4 * 512:(n4 + 1) * 512],
                                wsb[:, kc, blk * 128:(blk + 1) * 128],
                                X2[:, kc, n4 * 512:(n4 + 1) * 512],
                                start=(kc == 0), stop=(kc == 3))
                    nc.vector.tensor_scalar_add(dst, pq[:], bias)

                # ---- V transpose: contiguous store to DRAM, then 16
                # canonical 2D xbar transposes ([256, 128] -> [128, 256]),
                # then one strided re-layout copy into the 65-col head
                # groups (col 64 of each group = ones for softmax sums) ----
                for blk in range(2):
                    nc.sync.dma_start(v_dram[blk * 128:(blk + 1) * 128, :],
                                      Vc[:, blk, :])
                Vtb = work.tile([128, 16, 256], BF16, tag="vtb", name="Vtb",
                                bufs=1)
                for t in range(16):
                    nc.sync.dma_start_transpose(
                        Vtb[:, t, :], v_dram[:, t * 128:(t + 1) * 128])
                Vt = acts.tile([128, 16, 260], BF16)
                Vt4 = Vt[:].rearrange("p t (h m) -> p t h m", m=65)
                nc.vector.tensor_copy(
                    Vt4[:, :, :, 0:64],
                    Vtb[:].rearrange("p t (h m) -> p t h m", m=64))
                nc.vector.tensor_copy(
                    Vt4[:, :, :, 64],
                    ones_sb.rearrange("p (t h) -> p t h", t=16))
                # ---- attention ----
                OT = work.tile([128, 2, L], F32R, tag="vo", name="OT",
                               bufs=1)
                rr = work.tile([1, 4, 2048], F32, name="rr", bufs=1)
                for u in range(HPC):
                    blk, poff = u // 2, 64 * (u % 2)
                    pvp = ps_b.tile([128, 2048], F32, tag="pv", name="pvp")
                    # software-pipelined: PV runs one round behind scores+exp
                    P_prev = None
                    for lk in range(17):
                        P_cur = None
                        if lk >= 1:
                            lp = lk - 1
                            for c2 in range(4):
                                nc.tensor.matmul(
                                    pvp[0:65, c2 * 512:(c2 + 1) * 512],
                                    Vt[:, lp, u * 65:u * 65 + 65],
                                    P_prev[:, c2 * 512:(c2 + 1) * 512],
                                    start=(lp == 0), stop=(lp == 15))
                        if lk < 16:
                            S = ps_a.tile([128, 2048], F32, tag="s", name="S")
                            for c2 in range(4):
                                c0 = c2 * 512
                                nc.tensor.matmul(
                                    S[:, c0:c0 + 512],
                                    KT[poff:poff + 64, blk,
                                       lk * 128:(lk + 1) * 128],
                                    QT[poff:poff + 64, blk, c0:c0 + 512],
                                    start=True, stop=True)
                            P_cur = work.tile([128, 2048], BF16, tag="P",
                                              name="P", bufs=2)
                            nc.scalar.activation(
                                P_cur[:], S[:], AF.Exp,
                                scale=float(1.0 / np.sqrt(D)))
                        P_prev = P_cur
                    # per-head: softmax sums reciprocal + raw copy; the
                    # broadcasted divide happens once for all heads below
                    nc.vector.reciprocal(rr[:, u, :], pvp[64:65, :])
                    nc.vector.tensor_copy(OT[poff:poff + 64, blk, :],
                                          pvp[0:64, :])
                # batched normalization: one DRAM round trip broadcasts
                # rr[head, lq] across the 64 d-partitions of each head
                nc.sync.dma_start(r_dram[:], rr[:])
                src = r_dram[:]
                Rb = work.tile([128, 2, 2048], F32, name="Rb", bufs=1)
                for hl in range(2):
                    rbc = bass.AP(tensor=src.tensor,
                                  offset=src.offset + hl * 2048,
                                  ap=[[0, 64], [4096, 2], [1, 2048]])
                    nc.sync.dma_start(Rb[hl * 64:(hl + 1) * 64, :, :], rbc)
                nc.vector.tensor_mul(OT[:], OT[:].bitcast(F32), Rb[:])

                # ---- projection (partial over this core's 256 channels) ----
                for mt in range(4):
                    pool = ps_a if mt % 2 == 0 else ps_b
                    tag = "s" if mt % 2 == 0 else "pv"
                    po = pool.tile([128, 2048], F32, tag=tag, name="po")
                    for n4 in range(4):
                        for kc in range(2):
                            nc.tensor.matmul(
                                po[:, n4 * 512:(n4 + 1) * 512],
                                wp_sb[:, kc, mt * 128:(mt + 1) * 128],
                                OT[:, kc, n4 * 512:(n4 + 1) * 512],
                                start=(kc == 0), stop=(kc == 1))
                    ot = work.tile([128, 2048], F32, tag="ot", name="ot",
                                   bufs=2)
                    nc.vector.tensor_copy(ot[:], po[:])
                    nc.sync.dma_start(out_d[:, mt, :], ot[:])

        ctx_lp.__exit__(None, None, None)

    if apply_split:
        split_waits(nc, drop_own=drop_own)
    return nc


_CACHE = {}


def _get_nc():
    if "nc" not in _CACHE:
        _CACHE["nc"] = build_nc()
    return _CACHE["nc"]


def _core_inputs(i, x, gamma, beta, w_qkv, b_qkv, w_proj, b_proj):
    b, j0 = i // 2, i % 2
    heads = [HPC * j0 + k for k in range(HPC)]
    # Q/K row order: blk-major, within blk: head pair x d
    qidx = np.array([heads[blk * 2 + p // 64] * 64 + p % 64
                     for blk in range(2) for p in range(128)])
    kidx = qidx + C
    vidx = np.array([2 * C + heads[n // 64] * 64 + n % 64 for n in range(256)])
    pcol = np.array([heads[cc // 64] * 64 + cc % 64 for cc in range(256)])

    f32 = np.float32
    wq_a = np.ascontiguousarray(
        w_qkv[qidx].T.reshape(4, 128, 256).transpose(1, 0, 2)).astype(f32)
    wk_a = np.ascontiguousarray(
        w_qkv[kidx].T.reshape(4, 128, 256).transpose(1, 0, 2)).astype(f32)
    wv_a = np.ascontiguousarray(
        w_qkv[vidx].T.reshape(4, 128, 256).transpose(1, 0, 2)).astype(f32)
    gG = (np.arange(128)[:, None] // 16 == np.arange(8)[None, :]).astype(f32)
    sm = np.empty((128, 2062), f32)
    sm[:, 0:2] = b_qkv[qidx].reshape(2, 128).T
    sm[:, 2:4] = b_qkv[kidx].reshape(2, 128).T
    sm[:, 4:8] = gamma.reshape(4, 128).T
    sm[:, 8:12] = beta.reshape(4, 128).T
    sm[:, 12:14] = b_qkv[vidx].reshape(2, 128).T
    sm[:, 14:2062] = 0.0
    m = {
        "x_in": np.ascontiguousarray(
            x[b].reshape(4, 128, L).transpose(1, 0, 2)).astype(f32),
        "wqkv": np.concatenate([wq_a, wk_a, wv_a], axis=2),
        "wp": np.concatenate([
            np.ascontiguousarray(
                w_proj[:, pcol].T.reshape(2, 128, 512).transpose(1, 0, 2)
            ).astype(f32),
            np.stack([gG, np.zeros_like(gG)], axis=1),
            np.stack([np.ones((128, 64), f32),
                      np.zeros((128, 64), f32)], axis=1)], axis=2),
        "sm": sm,
        "gG2": (np.arange(8)[:, None]
                == np.arange(128)[None, :] // 16).astype(f32),
    }
    return m


def kernel(x, gamma, beta, w_qkv, b_qkv, w_proj, b_proj, _trace=False):
    x = np.asarray(x, dtype=np.float32)
    gamma = np.asarray(gamma, dtype=np.float32)
    beta = np.asarray(beta, dtype=np.float32)
    w_qkv = np.asarray(w_qkv, dtype=np.float32)
    b_qkv = np.asarray(b_qkv, dtype=np.float32)
    w_proj = np.asarray(w_proj, dtype=np.float32)
    b_proj = np.asarray(b_proj, dtype=np.float32)

    nc = _get_nc()
    in_maps = [_core_inputs(i, x, gamma, beta, w_qkv, b_qkv, w_proj, b_proj)
               for i in range(NCORES)]
    res = run_bass_kernel_spmd(nc, in_maps, list(range(NCORES)),
                               trace=_trace)
    out = np.empty((B, C, L), dtype=np.float32)
    for b in range(B):
        acc = x[b] + b_proj[:, None]
        for j0 in range(2):
            part = res.results[2 * b + j0]["out"]  # [128, 4, L]
            acc = acc + part.transpose(1, 0, 2).reshape(C, L)
        out[b] = acc
    if _trace:
        return out, res
    return out
